# revision 15
# baseline (speedup 1.0000x reference)
"""Bezier-to-image Gaussian splat kernel for Trainium2 (8 NeuronCores).

Reference computation (per sample b of 256):
    T = warped cubic Bernstein basis (30, 4)
    points = einsum('nk,blkc->blnc', T, x.reshape(B,160,4,2))   # (B,160,30,2)
    gx[b,l,i,n] = exp(-(i/60 - X[b,l,n])^2 / 2e-4)
    out[b,i,j]  = min(sum_{l,n} gx[b,l,i,n]*gy[b,l,j,n], 1)     # (B,60,60)

Strategy: pure data parallel, 32 samples per core.  Per sample the 4800
bezier points are processed in 40 chunks of 120 points (partition dim);
d[p,i] = i - 60*X_p is built by DVE tensor_scalar ops against a constant
iota row (fp16, 4x mode), the Gaussian is evaluated on the Scalar engine
(Derivative_Erf LUT = 2/sqrt(pi)*exp(-x^2) in a single batched pass, or
Square+Exp fallback), and the 60x60 image is accumulated on the Tensor
engine as sum_c GxT_c^T @ GyT_c into one PSUM bank.
"""

import math
import os

import numpy as np
import orjson

import bass_rust
import concourse.bass as bass
import concourse.mybir as mybir
import concourse.tile as tile
from concourse.bass_utils import run_bass_kernel_spmd

B, L, N, W = 256, 160, 30, 60
NCORES = 8
BC = B // NCORES          # samples per core
ALPHA = 2e-4
KEXP = 1.0 / (W * W * ALPHA)          # exponent scale in cell units: 1/0.72
SDERF = math.sqrt(KEXP)               # Derivative_Erf input scale
DERF_FIX = math.pi / 4.0              # undo (2/sqrt(pi))^2 from Derivative_Erf
CHUNKS = 40                           # 4 curves x 30 samples per chunk
PTS = 128                             # chunk partition dim: p = 32*lg + n, rows
                                      # n in {30,31} of each strip are dead
CW = 60                               # width of one chunk's band (= W)
R_HOLE = -60.0                        # r for dead rows -> d in [60,119] -> g=0

USE_DERF = os.environ.get("KERNEL_USE_DERF", "1") == "1"

LAST_RESULTS = None  # test harness reads profiling info from here


def _basis_T() -> np.ndarray:
    t = np.arange(N, dtype=np.float32) / np.float32(N)
    t = 2 * t**3 - 3 * t**2 + 2 * t
    t_3_0 = t**3
    t_2_1 = t**2 - t_3_0
    t_1_2 = t_3_0 - 2 * t**2 + t
    t_0_3 = (1 - t) ** 3
    return np.stack([t_3_0, 3 * t_2_1, 3 * t_1_2, t_0_3], axis=1).astype(np.float32)


def _legalize_waits(nc, max_waits: int = 1):
    """Walrus rejects engine instructions carrying more than ~1 sync wait
    ("Too many sync wait commands").  Hoist excess waits onto same-engine
    Drain instructions inserted immediately before the offender (the same
    carrier the Tile epilogue barrier uses, observed with up to 7 waits)."""
    js = orjson.loads(mybir.module_to_json_bytes(nc.m))
    ctr = 0
    for f in js["functions"]:
        for bb in f["blocks"]:
            out = []
            changed = False
            for inst in bb["instructions"]:
                si = inst.get("sync_info")
                waits = si.get("on_wait") if si else None
                if waits and len(waits) > max_waits:
                    keep = waits[:max_waits]
                    for w in waits[max_waits:]:
                        ctr += 1
                        out.append({
                            "debug": inst.get("debug", 0),
                            "engine": inst["engine"],
                            "ins": [], "outs": [],
                            "name": f"waitfix-{ctr}",
                            "opcode": "Drain",
                            "sync_info": {"on_update": [], "on_wait": [w]},
                        })
                    si["on_wait"] = keep
                    changed = True
                out.append(inst)
            if changed:
                bb["instructions"] = out
    if ctr:
        nc.m = bass_rust.module_from_json_bytes(orjson.dumps(js))
    return ctr


def build_program(use_derf: bool = USE_DERF, legalize: bool = True):
    f32 = mybir.dt.float32
    f16 = mybir.dt.float16

    nc = bass.Bass("TRN2", target_bir_lowering=False, debug=False)

    x_t = nc.dram_tensor("x", [BC, L, 8], f32, kind="ExternalInput")
    y_t = nc.dram_tensor("y", [BC, W, W], f32, kind="ExternalOutput")

    # (5, 32) stationary operand: r[m] = sum_k TscT[k,m]*ctrl[k] = 60*X for
    # m<30; row 4 contracts against a constant-ones row so the two dead
    # partitions of each 32-strip get r = R_HOLE (-> g = 0) with no memset.
    tsc_np = np.zeros((5, 32), dtype=np.float32)
    tsc_np[:4, :N] = (W * _basis_T()).T
    tsc_np[4, N:] = R_HOLE
    tsc_d = nc.inline_tensor(tsc_np, name="tscT")
    iota_np = np.tile(np.arange(CW, dtype=np.float16), (PTS, 1))  # (128, 60)
    iota_d = nc.inline_tensor(iota_np, name="iota60")

    with tile.TileContext(nc) as tc, tc.tile_pool(name="const", bufs=1) as cpool, \
            tc.tile_pool(name="ctrl", bufs=1) as ctrl_pool, \
            tc.tile_pool(name="outp", bufs=1) as out_pool, \
            tc.tile_pool(name="rwork", bufs=2) as rpool, \
            tc.tile_pool(name="band", bufs=2) as band_pool, \
            tc.tile_pool(name="rpsum", bufs=2, space="PSUM") as rps_pool, \
            tc.tile_pool(name="imgpsum", bufs=2, space="PSUM") as img_pool:

        # Prologue: DMA loads land in staging tiles; DVE copies them into the
        # tiles PE reads.  PE LDWEIGHTS tolerates very few sync waits, and
        # Tile emits per-DMA-queue waits with no cross-engine transitivity —
        # staging through DVE leaves PE with a single (merged) DVE wait.
        tsc0 = cpool.tile([5, 32], f32, tag="tsc0")
        nc.sync.dma_start(tsc0[:], tsc_d.ap())
        tsc = cpool.tile([5, 32], f32, tag="tsc")
        nc.vector.tensor_copy(tsc[:], tsc0[:])
        iot = cpool.tile([PTS, CW], f16, tag="iota")
        nc.sync.dma_start(iot[:], iota_d.ap())

        # control points: partition k (4) + ones row (4), free = (b, l, coord)
        ct0 = ctrl_pool.tile([4, BC * 2 * L], f32, tag="ct0")
        nc.sync.dma_start(
            ct0[:].rearrange("k (b l c) -> k b l c", b=BC, c=2),
            x_t.ap().rearrange("b l (k c) -> k b l c", k=4),
        )
        ct = ctrl_pool.tile([5, BC * 2 * L], f32, tag="ct")
        nc.vector.memset(ct[:], 1.0)          # row 4 stays all-ones
        nc.vector.tensor_copy(ct[0:4, :], ct0[:])
        ct_v = ct[:].rearrange("k (b c g co) -> k b c g co", b=BC, c=CHUNKS, co=2)

        # all 32 output images live here until the final DMA
        out_all = out_pool.tile([W, BC * W], f32, tag="oall")

        for b in range(BC):
            # ---- r = 60 * point coords, layout [p=(lg,n), (chunk, coord)] ----
            r_ps = rps_pool.tile([PTS, 2 * CHUNKS], f32, tag="rps")
            for lg in range(4):
                nc.tensor.matmul(
                    r_ps[32 * lg : 32 * lg + 32, :],
                    lhsT=tsc[:],
                    rhs=ct_v[:, b : b + 1, :, lg : lg + 1, :],
                    start=True,
                    stop=True,
                    tile_position=(0, 32 * lg),
                )
            r_sb = rpool.tile([PTS, 2 * CHUNKS], f32, tag="rsb")
            nc.vector.tensor_copy(r_sb[:], r_ps[:])

            # ---- banded distance + Gaussian, fp16; x in cols [0,2400),
            # y in cols [2400,4800) so one ACT covers both ----
            dd = band_pool.tile([PTS, 2 * CHUNKS * CW], f16, tag="dd")
            yo = CHUNKS * CW
            for c in range(CHUNKS):
                nc.vector.tensor_scalar_sub(
                    dd[:, CW * c : CW * (c + 1)], iot[:], r_sb[:, 2 * c : 2 * c + 1]
                )
                nc.vector.tensor_scalar_sub(
                    dd[:, yo + CW * c : yo + CW * (c + 1)],
                    iot[:], r_sb[:, 2 * c + 1 : 2 * c + 2],
                )
            gg = band_pool.tile([PTS, 2 * CHUNKS * CW], f16, tag="gg")
            if use_derf:
                nc.scalar.activation(
                    gg[:], dd[:],
                    mybir.ActivationFunctionType.Derivative_Erf,
                    bias=0.0, scale=SDERF,
                )
            else:
                nc.vector.tensor_mul(gg[:], dd[:], dd[:])
                nc.scalar.activation(
                    gg[:], gg[:],
                    mybir.ActivationFunctionType.Exp,
                    bias=0.0, scale=-KEXP,
                )

            # ---- image accumulation: sum_c GxT_c^T @ GyT_c ----
            img = img_pool.tile([W, W], f32, tag="img")
            for c in range(CHUNKS):
                nc.tensor.matmul(
                    img[:],
                    lhsT=gg[:, CW * c : CW * c + W],
                    rhs=gg[:, yo + CW * c : yo + CW * c + W],
                    start=(c == 0),
                    stop=(c == CHUNKS - 1),
                )

            # ---- min(scale*img, 1) -> staging ----
            nc.vector.tensor_scalar(
                out_all[:, W * b : W * (b + 1)],
                img[:],
                DERF_FIX if use_derf else 1.0,
                1.0,
                mybir.AluOpType.mult,
                mybir.AluOpType.min,
            )

        nc.sync.dma_start(
            y_t.ap().rearrange("b i j -> i b j"),
            out_all[:].rearrange("i (b j) -> i b j", b=BC),
        )

    if legalize:
        _legalize_waits(nc)
    return nc


_PROGRAM = None


def kernel(x: np.ndarray, _trace: bool = False) -> np.ndarray:
    global _PROGRAM, LAST_RESULTS
    assert x.shape == (B, L, 8) and x.dtype == np.float32, (x.shape, x.dtype)
    if _PROGRAM is None:
        _PROGRAM = build_program()
    nc = _PROGRAM
    shards = np.split(np.ascontiguousarray(x), NCORES, axis=0)
    in_maps = [{"x": s} for s in shards]
    res = run_bass_kernel_spmd(nc, in_maps, list(range(NCORES)), trace=_trace)
    LAST_RESULTS = res
    return np.concatenate([res.results[i]["y"] for i in range(NCORES)], axis=0)


# revision 16
# speedup vs baseline: 1.6114x; 1.6114x over previous
"""Bezier-to-image Gaussian splat kernel for Trainium2 (8 NeuronCores).

Reference computation (per sample b of 256):
    T = warped cubic Bernstein basis (30, 4)
    points = einsum('nk,blkc->blnc', T, x.reshape(B,160,4,2))   # (B,160,30,2)
    gx[b,l,i,n] = exp(-(i/60 - X[b,l,n])^2 / 2e-4)
    out[b,i,j]  = min(sum_{l,n} gx[b,l,i,n]*gy[b,l,j,n], 1)     # (B,60,60)

Strategy: pure data parallel, 32 samples per core.  Per sample the 4800
bezier points are processed in 40 chunks of 120 points (partition dim);
d[p,i] = i - 60*X_p is built by DVE tensor_scalar ops against a constant
iota row (fp16, 4x mode), the Gaussian is evaluated on the Scalar engine
(Derivative_Erf LUT = 2/sqrt(pi)*exp(-x^2) in a single batched pass, or
Square+Exp fallback), and the 60x60 image is accumulated on the Tensor
engine as sum_c GxT_c^T @ GyT_c into one PSUM bank.
"""

import math
import os

import numpy as np
import orjson

import bass_rust
import concourse.bass as bass
import concourse.mybir as mybir
import concourse.tile as tile
from concourse.bass_utils import run_bass_kernel_spmd

B, L, N, W = 256, 160, 30, 60
NCORES = 8
BC = B // NCORES          # samples per core
ALPHA = 2e-4
KEXP = 1.0 / (W * W * ALPHA)          # exponent scale in cell units: 1/0.72
SDERF = math.sqrt(KEXP)               # Derivative_Erf input scale
DERF_FIX = math.pi / 4.0              # undo (2/sqrt(pi))^2 from Derivative_Erf
CHUNKS = 40                           # 4 curves x 30 samples per chunk
PTS = 128                             # chunk partition dim: p = 32*lg + n, rows
                                      # n in {30,31} of each strip are dead
CW = 60                               # width of one chunk's band (= W)
R_HOLE = -60.0                        # r for dead rows -> d in [60,119] -> g=0

USE_DERF = os.environ.get("KERNEL_USE_DERF", "1") == "1"

LAST_RESULTS = None  # test harness reads profiling info from here


def _basis_T() -> np.ndarray:
    t = np.arange(N, dtype=np.float32) / np.float32(N)
    t = 2 * t**3 - 3 * t**2 + 2 * t
    t_3_0 = t**3
    t_2_1 = t**2 - t_3_0
    t_1_2 = t_3_0 - 2 * t**2 + t
    t_0_3 = (1 - t) ** 3
    return np.stack([t_3_0, 3 * t_2_1, 3 * t_1_2, t_0_3], axis=1).astype(np.float32)


def _legalize_waits(nc, max_waits: int = 1):
    """Walrus rejects engine instructions carrying more than ~1 sync wait
    ("Too many sync wait commands").  Hoist excess waits onto same-engine
    Drain instructions inserted immediately before the offender (the same
    carrier the Tile epilogue barrier uses, observed with up to 7 waits)."""
    js = orjson.loads(mybir.module_to_json_bytes(nc.m))
    ctr = 0
    for f in js["functions"]:
        for bb in f["blocks"]:
            out = []
            changed = False
            for inst in bb["instructions"]:
                si = inst.get("sync_info")
                waits = si.get("on_wait") if si else None
                if waits and len(waits) > max_waits:
                    keep = waits[:max_waits]
                    for w in waits[max_waits:]:
                        ctr += 1
                        out.append({
                            "debug": inst.get("debug", 0),
                            "engine": inst["engine"],
                            "ins": [], "outs": [],
                            "name": f"waitfix-{ctr}",
                            "opcode": "Drain",
                            "sync_info": {"on_update": [], "on_wait": [w]},
                        })
                    si["on_wait"] = keep
                    changed = True
                out.append(inst)
            if changed:
                bb["instructions"] = out
    if ctr:
        nc.m = bass_rust.module_from_json_bytes(orjson.dumps(js))
    return ctr


def build_program(use_derf: bool = USE_DERF, legalize: bool = True):
    f32 = mybir.dt.float32
    f16 = mybir.dt.float16

    nc = bass.Bass("TRN2", target_bir_lowering=False, debug=False)

    x_t = nc.dram_tensor("x", [BC, L, 8], f32, kind="ExternalInput")
    y_t = nc.dram_tensor("y", [BC, W, W], f32, kind="ExternalOutput")

    # (5, 32) stationary operand: r[m] = sum_k TscT[k,m]*ctrl[k] = 60*X for
    # m<30; row 4 contracts against a constant-ones row so the two dead
    # partitions of each 32-strip get r = R_HOLE (-> g = 0) with no memset.
    tsc_np = np.zeros((5, 32), dtype=np.float32)
    tsc_np[:4, :N] = (W * _basis_T()).T
    tsc_np[4, N:] = R_HOLE
    tsc_d = nc.inline_tensor(tsc_np, name="tscT")
    iota_np = np.tile(np.arange(CW, dtype=np.float16), (PTS, 1))  # (128, 60)
    iota_d = nc.inline_tensor(iota_np, name="iota60")

    with tile.TileContext(nc) as tc, tc.tile_pool(name="const", bufs=1) as cpool, \
            tc.tile_pool(name="ctrl", bufs=1) as ctrl_pool, \
            tc.tile_pool(name="outp", bufs=1) as out_pool, \
            tc.tile_pool(name="rwork", bufs=2) as rpool, \
            tc.tile_pool(name="band", bufs=2) as band_pool, \
            tc.tile_pool(name="rpsum", bufs=2, space="PSUM") as rps_pool, \
            tc.tile_pool(name="imgpsum", bufs=2, space="PSUM") as img_pool:

        # Prologue: DMA loads land in staging tiles; DVE copies them into the
        # tiles PE reads.  PE LDWEIGHTS tolerates very few sync waits, and
        # Tile emits per-DMA-queue waits with no cross-engine transitivity —
        # staging through DVE leaves PE with a single (merged) DVE wait.
        tsc0 = cpool.tile([5, 32], f32, tag="tsc0")
        nc.sync.dma_start(tsc0[:], tsc_d.ap())
        tsc = cpool.tile([5, 32], f32, tag="tsc")
        nc.vector.tensor_copy(tsc[:], tsc0[:])
        iot = cpool.tile([PTS, CW], f16, tag="iota")
        nc.sync.dma_start(iot[:], iota_d.ap())

        # control points: partition k (4) + ones row (4), free = (b, l, coord)
        ct0 = ctrl_pool.tile([4, BC * 2 * L], f32, tag="ct0")
        nc.sync.dma_start(
            ct0[:].rearrange("k (b l c) -> k b l c", b=BC, c=2),
            x_t.ap().rearrange("b l (k c) -> k b l c", k=4),
        )
        ct = ctrl_pool.tile([5, BC * 2 * L], f32, tag="ct")
        nc.vector.memset(ct[:], 1.0)          # row 4 stays all-ones
        nc.vector.tensor_copy(ct[0:4, :], ct0[:])
        ct_v = ct[:].rearrange("k (b c g co) -> k b c g co", b=BC, c=CHUNKS, co=2)

        # all 32 output images live here until the final DMA
        out_all = out_pool.tile([W, BC * W], f32, tag="oall")

        for b in range(BC):
            # ---- r = 60 * point coords, layout [p=(lg,n), (chunk, coord)] ----
            r_ps = rps_pool.tile([PTS, 2 * CHUNKS], f32, tag="rps")
            for lg in range(4):
                nc.tensor.matmul(
                    r_ps[32 * lg : 32 * lg + 32, :],
                    lhsT=tsc[:],
                    rhs=ct_v[:, b : b + 1, :, lg : lg + 1, :],
                    start=True,
                    stop=True,
                    tile_position=(0, 32 * lg),
                )
            r_sb = rpool.tile([PTS, 2 * CHUNKS], f32, tag="rsb")
            nc.vector.tensor_copy(r_sb[:], r_ps[:])

            # ---- banded distance + Gaussian, fp16; x in cols [0,2400),
            # y in cols [2400,4800) so one ACT covers both.  One batched
            # tensor_tensor per side (broadcast APs) instead of 80 tiny
            # tensor_scalar ops: the ~60-cycle DVE fixed cost per op was
            # the kernel's bottleneck (measured ~150ns x 2560 ops).
            dd = band_pool.tile([PTS, 2 * CHUNKS * CW], f16, tag="dd")
            yo = CHUNKS * CW
            iota_b = iot[:].rearrange("p (o w) -> p o w", o=1).broadcast_to(
                [PTS, CHUNKS, CW]
            )
            r_v = r_sb[:].rearrange("p (c co) -> p c co", co=2)
            for side in range(2):
                nc.vector.tensor_tensor(
                    dd[:, yo * side : yo * (side + 1)].rearrange(
                        "p (c w) -> p c w", w=CW
                    ),
                    iota_b,
                    r_v[:, :, side : side + 1].broadcast_to([PTS, CHUNKS, CW]),
                    mybir.AluOpType.subtract,
                )
            gg = band_pool.tile([PTS, 2 * CHUNKS * CW], f16, tag="gg")
            if use_derf:
                nc.scalar.activation(
                    gg[:], dd[:],
                    mybir.ActivationFunctionType.Derivative_Erf,
                    bias=0.0, scale=SDERF,
                )
            else:
                nc.vector.tensor_mul(gg[:], dd[:], dd[:])
                nc.scalar.activation(
                    gg[:], gg[:],
                    mybir.ActivationFunctionType.Exp,
                    bias=0.0, scale=-KEXP,
                )

            # ---- image accumulation: sum_c GxT_c^T @ GyT_c ----
            img = img_pool.tile([W, W], f32, tag="img")
            for c in range(CHUNKS):
                nc.tensor.matmul(
                    img[:],
                    lhsT=gg[:, CW * c : CW * c + W],
                    rhs=gg[:, yo + CW * c : yo + CW * c + W],
                    start=(c == 0),
                    stop=(c == CHUNKS - 1),
                )

            # ---- min(scale*img, 1) -> staging ----
            nc.vector.tensor_scalar(
                out_all[:, W * b : W * (b + 1)],
                img[:],
                DERF_FIX if use_derf else 1.0,
                1.0,
                mybir.AluOpType.mult,
                mybir.AluOpType.min,
            )

        nc.sync.dma_start(
            y_t.ap().rearrange("b i j -> i b j"),
            out_all[:].rearrange("i (b j) -> i b j", b=BC),
        )

    if legalize:
        _legalize_waits(nc)
    return nc


_PROGRAM = None


def kernel(x: np.ndarray, _trace: bool = False) -> np.ndarray:
    global _PROGRAM, LAST_RESULTS
    assert x.shape == (B, L, 8) and x.dtype == np.float32, (x.shape, x.dtype)
    if _PROGRAM is None:
        _PROGRAM = build_program()
    nc = _PROGRAM
    shards = np.split(np.ascontiguousarray(x), NCORES, axis=0)
    in_maps = [{"x": s} for s in shards]
    res = run_bass_kernel_spmd(nc, in_maps, list(range(NCORES)), trace=_trace)
    LAST_RESULTS = res
    return np.concatenate([res.results[i]["y"] for i in range(NCORES)], axis=0)


# revision 18
# speedup vs baseline: 1.7967x; 1.1150x over previous
"""Bezier-to-image Gaussian splat kernel for Trainium2 (8 NeuronCores).

Reference computation (per sample b of 256):
    T = warped cubic Bernstein basis (30, 4)
    points = einsum('nk,blkc->blnc', T, x.reshape(B,160,4,2))   # (B,160,30,2)
    gx[b,l,i,n] = exp(-(i/60 - X[b,l,n])^2 / 2e-4)
    out[b,i,j]  = min(sum_{l,n} gx[b,l,i,n]*gy[b,l,j,n], 1)     # (B,60,60)

Strategy: pure data parallel, 32 samples per core.  Per sample the 4800
bezier points are processed in 40 chunks of 120 points (partition dim);
d[p,i] = i - 60*X_p is built by DVE tensor_scalar ops against a constant
iota row (fp16, 4x mode), the Gaussian is evaluated on the Scalar engine
(Derivative_Erf LUT = 2/sqrt(pi)*exp(-x^2) in a single batched pass, or
Square+Exp fallback), and the 60x60 image is accumulated on the Tensor
engine as sum_c GxT_c^T @ GyT_c into one PSUM bank.
"""

import math
import os

import numpy as np
import orjson

import bass_rust
import concourse.bass as bass
import concourse.mybir as mybir
import concourse.tile as tile
from concourse.bass_utils import run_bass_kernel_spmd

B, L, N, W = 256, 160, 30, 60
NCORES = 8
BC = B // NCORES          # samples per core
ALPHA = 2e-4
KEXP = 1.0 / (W * W * ALPHA)          # exponent scale in cell units: 1/0.72
SDERF = math.sqrt(KEXP)               # Derivative_Erf input scale
DERF_FIX = math.pi / 4.0              # undo (2/sqrt(pi))^2 from Derivative_Erf
CHUNKS = 40                           # 4 curves x 30 samples per chunk
PTS = 128                             # chunk partition dim: p = 32*lg + n, rows
                                      # n in {30,31} of each strip are dead
CW = 60                               # width of one chunk's band (= W)
R_HOLE = -60.0                        # r for dead rows -> d in [60,119] -> g=0

USE_DERF = os.environ.get("KERNEL_USE_DERF", "1") == "1"

LAST_RESULTS = None  # test harness reads profiling info from here


def _basis_T() -> np.ndarray:
    t = np.arange(N, dtype=np.float32) / np.float32(N)
    t = 2 * t**3 - 3 * t**2 + 2 * t
    t_3_0 = t**3
    t_2_1 = t**2 - t_3_0
    t_1_2 = t_3_0 - 2 * t**2 + t
    t_0_3 = (1 - t) ** 3
    return np.stack([t_3_0, 3 * t_2_1, 3 * t_1_2, t_0_3], axis=1).astype(np.float32)


def _legalize_waits(nc, max_waits: int = 1):
    """Walrus rejects engine instructions carrying more than ~1 sync wait
    ("Too many sync wait commands").  Hoist excess waits onto same-engine
    Drain instructions inserted immediately before the offender (the same
    carrier the Tile epilogue barrier uses, observed with up to 7 waits)."""
    js = orjson.loads(mybir.module_to_json_bytes(nc.m))
    ctr = 0
    for f in js["functions"]:
        for bb in f["blocks"]:
            out = []
            changed = False
            for inst in bb["instructions"]:
                si = inst.get("sync_info")
                waits = si.get("on_wait") if si else None
                if waits and len(waits) > max_waits:
                    keep = waits[:max_waits]
                    for w in waits[max_waits:]:
                        ctr += 1
                        out.append({
                            "debug": inst.get("debug", 0),
                            "engine": inst["engine"],
                            "ins": [], "outs": [],
                            "name": f"waitfix-{ctr}",
                            "opcode": "Drain",
                            "sync_info": {"on_update": [], "on_wait": [w]},
                        })
                    si["on_wait"] = keep
                    changed = True
                out.append(inst)
            if changed:
                bb["instructions"] = out
    if ctr:
        nc.m = bass_rust.module_from_json_bytes(orjson.dumps(js))
    return ctr


def build_program(use_derf: bool = USE_DERF, legalize: bool = True):
    f32 = mybir.dt.float32
    f16 = mybir.dt.float16

    nc = bass.Bass("TRN2", target_bir_lowering=False, debug=False)

    x_t = nc.dram_tensor("x", [BC, L, 8], f32, kind="ExternalInput")
    y_t = nc.dram_tensor("y", [BC, W, W], f32, kind="ExternalOutput")

    # (5, 32) stationary operand: r[m] = sum_k TscT[k,m]*ctrl[k] = 60*X for
    # m<30; row 4 contracts against a constant-ones row so the two dead
    # partitions of each 32-strip get r = R_HOLE (-> g = 0) with no memset.
    tsc_np = np.zeros((5, 32), dtype=np.float32)
    tsc_np[:4, :N] = (W * _basis_T()).T
    tsc_np[4, N:] = R_HOLE
    tsc_d = nc.inline_tensor(tsc_np, name="tscT")
    iota_np = np.tile(np.arange(CW, dtype=np.float16), (PTS, 1))  # (128, 60)
    iota_d = nc.inline_tensor(iota_np, name="iota60")

    with tile.TileContext(nc) as tc, tc.tile_pool(name="const", bufs=1) as cpool, \
            tc.tile_pool(name="ctrl", bufs=1) as ctrl_pool, \
            tc.tile_pool(name="outp", bufs=1) as out_pool, \
            tc.tile_pool(name="rwork", bufs=2) as rpool, \
            tc.tile_pool(name="band", bufs=3) as band_pool, \
            tc.tile_pool(name="rpsum", bufs=2, space="PSUM") as rps_pool, \
            tc.tile_pool(name="imgpsum", bufs=2, space="PSUM") as img_pool:

        # Prologue: DMA loads land in staging tiles; DVE copies them into the
        # tiles PE reads.  PE LDWEIGHTS tolerates very few sync waits, and
        # Tile emits per-DMA-queue waits with no cross-engine transitivity —
        # staging through DVE leaves PE with a single (merged) DVE wait.
        tsc0 = cpool.tile([5, 32], f32, tag="tsc0")
        nc.sync.dma_start(tsc0[:], tsc_d.ap())
        tsc = cpool.tile([5, 32], f32, tag="tsc")
        nc.vector.tensor_copy(tsc[:], tsc0[:])
        iot = cpool.tile([PTS, CW], f16, tag="iota")
        nc.sync.dma_start(iot[:], iota_d.ap())

        # control points: partition k (4) + ones row (4), free = (b, l, coord).
        # Loaded in groups of 8 samples so compute overlaps the (descriptor-
        # heavy, ~20us/group) strided load instead of stalling ~80us upfront.
        GRP = 8
        ct = ctrl_pool.tile([5, BC * 2 * L], f32, tag="ct")
        nc.vector.memset(ct[:], 1.0)          # row 4 stays all-ones
        gsz = GRP * 2 * L
        for g in range(BC // GRP):
            ct0 = rpool.tile([4, gsz], f32, tag="ct0")
            nc.sync.dma_start(
                ct0[:].rearrange("k (b l c) -> k b l c", b=GRP, c=2),
                x_t.ap()[g * GRP : (g + 1) * GRP]
                .rearrange("b l (k c) -> k b l c", k=4),
            )
            nc.vector.tensor_copy(ct[0:4, g * gsz : (g + 1) * gsz], ct0[:])
        ct_v = ct[:].rearrange("k (b c g co) -> k b c g co", b=BC, c=CHUNKS, co=2)

        # all 32 output images live here until the final DMA
        out_all = out_pool.tile([W, BC * W], f32, tag="oall")

        for b in range(BC):
            # ---- r = 60 * point coords, layout [p=(lg,n), (chunk, coord)] ----
            r_ps = rps_pool.tile([PTS, 2 * CHUNKS], f32, tag="rps")
            for lg in range(4):
                nc.tensor.matmul(
                    r_ps[32 * lg : 32 * lg + 32, :],
                    lhsT=tsc[:],
                    rhs=ct_v[:, b : b + 1, :, lg : lg + 1, :],
                    start=True,
                    stop=True,
                    tile_position=(0, 32 * lg),
                )
            r_sb = rpool.tile([PTS, 2 * CHUNKS], f32, tag="rsb")
            nc.vector.tensor_copy(r_sb[:], r_ps[:])

            # ---- banded distance + Gaussian, fp16; x in cols [0,2400),
            # y in cols [2400,4800) so one ACT covers both.  One batched
            # tensor_tensor per side (broadcast APs) instead of 80 tiny
            # tensor_scalar ops: the ~60-cycle DVE fixed cost per op was
            # the kernel's bottleneck (measured ~150ns x 2560 ops).
            dd = band_pool.tile([PTS, 2 * CHUNKS * CW], f16, tag="dd")
            yo = CHUNKS * CW
            iota_b = iot[:].rearrange("p (o w) -> p o w", o=1).broadcast_to(
                [PTS, CHUNKS, CW]
            )
            r_v = r_sb[:].rearrange("p (c co) -> p c co", co=2)
            for side in range(2):
                nc.vector.tensor_tensor(
                    dd[:, yo * side : yo * (side + 1)].rearrange(
                        "p (c w) -> p c w", w=CW
                    ),
                    iota_b,
                    r_v[:, :, side : side + 1].broadcast_to([PTS, CHUNKS, CW]),
                    mybir.AluOpType.subtract,
                )
            gg = band_pool.tile([PTS, 2 * CHUNKS * CW], f16, tag="gg")
            if use_derf:
                nc.scalar.activation(
                    gg[:], dd[:],
                    mybir.ActivationFunctionType.Derivative_Erf,
                    bias=0.0, scale=SDERF,
                )
            else:
                nc.vector.tensor_mul(gg[:], dd[:], dd[:])
                nc.scalar.activation(
                    gg[:], gg[:],
                    mybir.ActivationFunctionType.Exp,
                    bias=0.0, scale=-KEXP,
                )

            # ---- image accumulation: sum_c GxT_c^T @ GyT_c ----
            img = img_pool.tile([W, W], f32, tag="img")
            for c in range(CHUNKS):
                nc.tensor.matmul(
                    img[:],
                    lhsT=gg[:, CW * c : CW * c + W],
                    rhs=gg[:, yo + CW * c : yo + CW * c + W],
                    start=(c == 0),
                    stop=(c == CHUNKS - 1),
                )

            # ---- min(scale*img, 1) -> staging ----
            nc.vector.tensor_scalar(
                out_all[:, W * b : W * (b + 1)],
                img[:],
                DERF_FIX if use_derf else 1.0,
                1.0,
                mybir.AluOpType.mult,
                mybir.AluOpType.min,
            )

        nc.sync.dma_start(
            y_t.ap().rearrange("b i j -> i b j"),
            out_all[:].rearrange("i (b j) -> i b j", b=BC),
        )

    if legalize:
        _legalize_waits(nc)
    return nc


_PROGRAM = None


def kernel(x: np.ndarray, _trace: bool = False) -> np.ndarray:
    global _PROGRAM, LAST_RESULTS
    assert x.shape == (B, L, 8) and x.dtype == np.float32, (x.shape, x.dtype)
    if _PROGRAM is None:
        _PROGRAM = build_program()
    nc = _PROGRAM
    shards = np.split(np.ascontiguousarray(x), NCORES, axis=0)
    in_maps = [{"x": s} for s in shards]
    res = run_bass_kernel_spmd(nc, in_maps, list(range(NCORES)), trace=_trace)
    LAST_RESULTS = res
    return np.concatenate([res.results[i]["y"] for i in range(NCORES)], axis=0)
